# revision 17
# baseline (speedup 1.0000x reference)
"""Bahdanau attention weights kernel for 8 Trainium2 NeuronCores.

Reference computation (per full input):
    proj_enc = encoder_output @ W1_w + W1_b            # [B,S,U]
    proj_h   = last_layer_h_n @ W2_w + W2_b            # [B,1,U]
    score    = tanh(proj_enc + proj_h) @ V_w + V_b     # [B,S,1]
    out      = softmax(score, axis=1)                  # [B,S,1]

Sharding: data-parallel over batch. Each of the 8 cores gets B/8 batches;
weights are replicated; softmax is over the local sequence axis, so no
cross-core communication is needed.

Per-core layout strategy: keep U on partitions.
  - The host pre-transposes encoder_output to [H, tokens] so X^T tiles
    DMA straight into SBUF, and pre-folds the tiny bias chain
    (h_n @ W2 + W1_b + W2_b -> [b, U]) so no W2 traffic or bias matmuls
    hit the device at startup.
  - Main matmuls compute proj^T [u=128, t=512] in PSUM. The first `k8`
    rows of the H contraction run as fp8e4 DoubleRow matmuls (K=256 per
    MM at ~1.4x bf16 FLOP rate); the rest accumulate in bf16. k8=256
    keeps the softmax rel-err ~1.4e-2 (vs 2.2e-3 all-bf16, gate 2e-2).
  - tanh runs on the scalar engine reading PSUM with the folded bias as
    the per-partition bias operand.
  - The V contraction runs on the DVE: acc += V_ub (.) tanh_ub with V as
    a per-partition f32 scalar into an f32r accumulator; one f32r
    all-ones matmul per group sums it over its 128 partitions.
  - Softmax per batch is Exp(accum_out) / reciprocal / scale on the
    [1, 2048] score row (scores are bounded, so no max subtraction).
    The final scale is split across DVE+ACT halves to shorten the tail.
  - Startup: group-0 X chunks and W1 blocks DMA interleaved per h-block
    and group 0 runs hb-major over 4 PSUM banks per pass, so the PE
    streams real matmuls as each chunk pair lands; dummy matmuls warm
    the HAM clock first.
  - The last group contracts V on the PE directly (8 small f32r matmuls
    accumulating into the merge bank) so no DVE chain sits in the tail.
"""

import sys

for _p in ("/opt/trn_rl_repo", "/root/.axon_site/_ro/trn_rl_repo"):
    if _p not in sys.path:
        sys.path.append(_p)

import numpy as np

import concourse.bacc as bacc
import concourse.tile as tile
from concourse import mybir

F32 = mybir.dt.float32
F32R = mybir.dt.float32r
BF16 = mybir.dt.bfloat16
F8E4 = mybir.dt.float8e4
DR = mybir.MatmulPerfMode.DoubleRow

B, S, H, U = 32, 2048, 1024, 1024
N_CORES = 8
B_LOCAL = B // N_CORES  # 4
P = 128
T_GROUP = 512  # tokens per group (matmul moving dim)
K8 = 512       # rows of the H contraction done in fp8 DoubleRow


def build_kernel(b_local=B_LOCAL, s=S, h=H, u=U, k8=K8):
    """Build the per-core Bass program. Shape params must keep:
    s % T_GROUP == 0, h % 128 == 0, u % 128 == 0, k8 % 256 == 0."""
    nc = bacc.Bacc()

    n_tok = b_local * s
    n_groups = n_tok // T_GROUP
    groups_per_batch = s // T_GROUP
    KJ = k8 // 256           # DoubleRow matmuls per chain
    h_bf = h - k8
    HB = h_bf // P           # bf16 h blocks per chain
    UB = u // P              # u blocks
    PREFETCH = 5

    if k8:
        enc8 = nc.dram_tensor("enc8", [k8, n_tok], F8E4, kind="ExternalInput")
        w18 = nc.dram_tensor("W1_8", [k8, u], F8E4, kind="ExternalInput")
        enc8_v = enc8.ap().rearrange(
            "(kj j p) (g t) -> g p kj j t", p=P, j=2, t=T_GROUP)
        w18_v = w18.ap().rearrange("(kj j p) u -> kj p j u", p=P, j=2)
    enc = nc.dram_tensor("encoder_output", [h_bf, n_tok], BF16,
                         kind="ExternalInput")
    w1 = nc.dram_tensor("W1_w", [h_bf, u], BF16, kind="ExternalInput")
    bias_in = nc.dram_tensor("bias_in", [P, UB * b_local], F32,
                             kind="ExternalInput")
    vw = nc.dram_tensor("V_w", [u, 1], F32, kind="ExternalInput")
    vb = nc.dram_tensor("V_b", [1], F32, kind="ExternalInput")
    out = nc.dram_tensor("out", [b_local, s], F32, kind="ExternalOutput")

    encT_v = enc.ap().rearrange("(hb p) (g t) -> g p hb t", p=P, t=T_GROUP)
    w1_v = w1.ap().rearrange("(hb p) u -> hb p u", p=P)

    with tile.TileContext(nc) as tc:
        with (
            tc.tile_pool(name="consts", bufs=1) as consts,
            tc.tile_pool(name="wpool", bufs=1) as wpool,
            tc.tile_pool(name="x0pool", bufs=1) as x0pool,
            tc.tile_pool(name="xtpool", bufs=PREFETCH + 1) as xtpool,
            tc.tile_pool(name="thpool", bufs=4) as thpool,
            tc.tile_pool(name="scpool", bufs=3) as scpool,
            tc.tile_pool(name="rowpool", bufs=2) as rowpool,
            tc.tile_pool(name="smpool", bufs=2) as smpool,
            tc.tile_pool(name="psu", bufs=6, space="PSUM") as psu,
            tc.tile_pool(name="psmg", bufs=1, space="PSUM") as psmg,
            tc.tile_pool(name="pswm", bufs=1, space="PSUM") as pswm,
        ):
            # ---- warm-up fodder (no DMA dependencies) ---------------------
            # one ACT memzero readies both dummy operands (the scalar
            # engine's preamble finishes first); N=1 const-AP matmuls were
            # tried instead and hard-fault the exec unit (NRT 101)
            dummy_x = consts.tile([P, T_GROUP], BF16)
            nc.scalar.memzero(dummy_x)
            warm_ps = pswm.tile([P, T_GROUP], F32, tag="warm")

            def dummy_mm(n):
                for _ in range(n):
                    nc.tensor.matmul(warm_ps, lhsT=dummy_x[:, :P], rhs=dummy_x)

            dummy_mm(7)

            # ---- DMA: group-0 X chunks interleaved with W1 blocks ---------
            # (the critical ~2.6MB: each chunk pair unlocks 4 real matmuls)
            w18_sb = []
            x80 = None
            if k8:
                x80 = x0pool.tile([P, KJ, 2, T_GROUP], F8E4, tag="x8_0")
                for kj in range(KJ):
                    nc.sync.dma_start(out=x80[:, kj], in_=enc8_v[0][:, kj])
                    t8 = wpool.tile([P, 2, u], F8E4, tag=f"w18_{kj}")
                    nc.sync.dma_start(out=t8, in_=w18_v[kj])
                    w18_sb.append(t8)
            x0_tiles = []
            w1_sb = []
            for hb in range(HB):
                xt = x0pool.tile([P, T_GROUP], BF16, tag=f"x0_{hb}")
                nc.sync.dma_start(out=xt, in_=encT_v[0, :, hb, :])
                x0_tiles.append(xt)
                t1 = wpool.tile([P, u], BF16, tag=f"w1b_{hb}")
                nc.sync.dma_start(out=t1, in_=w1_v[hb])
                w1_sb.append(t1)

            # ---- small consts (ride the small-DMA ring) -------------------
            v_sb = consts.tile([P, UB], F32)
            nc.sync.dma_start(
                out=v_sb, in_=vw.ap().rearrange("(ub p) one -> p (ub one)", p=P)
            )
            vb_sb = consts.tile([1, 1], F32)
            nc.sync.dma_start(out=vb_sb, in_=vb.ap().rearrange("(a b) -> a b", a=1))
            bias_sb = consts.tile([P, UB, b_local], F32)
            nc.sync.dma_start(
                out=bias_sb,
                in_=bias_in.ap().rearrange("p (ub b) -> p ub b", b=b_local),
            )

            # all-ones column (f32r): one matmul sums the V-weighted tanh
            # accumulator over its 128 partitions (memset can't emit f32r,
            # so memset f32 and round through a copy)
            ones_f = consts.tile([P, 1], F32)
            nc.vector.memset(ones_f, 1.0)
            ones_sb = consts.tile([P, 1], F32R)
            nc.vector.tensor_copy(ones_sb, ones_f)
            # f32r copy of V for the last group's PE-side V contraction
            v_r = consts.tile([P, UB], F32R)
            nc.vector.tensor_copy(v_r, v_sb)

            # ---- X prefetch for groups 1..PREFETCH-1 ----------------------
            x_pending = {}

            def issue_x(g):
                x8t = None
                if k8:
                    x8t = xtpool.tile([P, KJ, 2, T_GROUP], F8E4, tag="x8")
                    nc.sync.dma_start(out=x8t, in_=enc8_v[g])
                xT = xtpool.tile([P, HB, T_GROUP], BF16, tag="xT")
                nc.sync.dma_start(out=xT, in_=encT_v[g])
                x_pending[g] = (x8t, xT)

            for g0 in range(1, min(PREFETCH, n_groups)):
                issue_x(g0)

            # ---- softmax state & helpers ----------------------------------
            state = {"sc_row": None, "esums": None, "pending": None}

            def finish_batch(pb):
                sc_row, esums = state["sc_row"], state["esums"]
                esum = smpool.tile([1, 1], F32, tag="esum")
                nc.vector.tensor_reduce(
                    esum, esums, axis=mybir.AxisListType.X,
                    op=mybir.AluOpType.add,
                )
                rec = smpool.tile([1, 1], F32, tag="rec")
                nc.vector.reciprocal(rec, esum)
                # scale the row split DVE/ACT by their measured rates (DVE
                # ~1.4 elem/ns vs ACT ~0.83 single-partition), sliced so
                # each output DMA starts as soon as its slice is scaled
                cut = (s * 21) // 32
                q = cut // 2
                for lo, hi in ((0, q), (q, cut)):
                    nc.vector.tensor_scalar_mul(
                        sc_row[:, lo:hi], sc_row[:, lo:hi], rec)
                    nc.sync.dma_start(out=out.ap()[pb : pb + 1, lo:hi],
                                      in_=sc_row[:, lo:hi])
                nc.scalar.mul(sc_row[:, cut:], sc_row[:, cut:], rec)
                nc.sync.dma_start(out=out.ap()[pb : pb + 1, cut:],
                                  in_=sc_row[:, cut:])

            def emit_exp(score_ps, pb, pgi):
                # score chunk -> exp incrementally per chunk (adds V_b).
                # scores are bounded (|score| <= sum|V_w|+|V_b| < 17), so
                # exp without max-subtraction is safe in fp32.
                if pgi == 0:
                    state["sc_row"] = rowpool.tile(
                        [1, s], F32, tag="scrow", name="sc_row")
                    state["esums"] = smpool.tile(
                        [1, groups_per_batch], F32, tag="esums", name="esums")
                sc_row, esums = state["sc_row"], state["esums"]
                nc.scalar.activation(
                    sc_row[:, pgi * T_GROUP : (pgi + 1) * T_GROUP], score_ps,
                    mybir.ActivationFunctionType.Exp,
                    bias=vb_sb,
                    accum_out=esums[:, pgi : pgi + 1],
                )
                if pgi == groups_per_batch - 1:
                    finish_batch(pb)

            def finish_pe(acc, pb, pgi):
                # merge: f32r ones-matmul reads the f32r accumulator
                # (a GPSIMD partition_all_reduce was tried instead: ~3.5us
                # per group and it stalls the PE ~1us/group — net loss)
                score_ps = psmg.tile([1, T_GROUP], F32, tag="mg")
                nc.tensor.matmul(score_ps, lhsT=ones_sb, rhs=acc)
                emit_exp(score_ps, pb, pgi)

            def chain_dr(pu, ub, x8t, kj, start):
                nc.tensor.matmul(
                    pu,
                    lhsT=w18_sb[kj][:, :, ub * P : (ub + 1) * P],
                    rhs=x8t[:, kj],
                    start=start, stop=False,
                    perf_mode=DR,
                )

            def chain_bf(pu, ub, xbt, hb):
                nc.tensor.matmul(
                    pu,
                    lhsT=w1_sb[hb][:, ub * P : (ub + 1) * P],
                    rhs=xbt[hb] if isinstance(xbt, list) else xbt[:, hb, :],
                    start=(k8 == 0 and hb == 0),
                    stop=(hb == HB - 1),
                )

            def tanh_dve(pu, ub, b, acc):
                th = thpool.tile([P, T_GROUP], BF16, tag="th")
                nc.scalar.activation(
                    th, pu,
                    mybir.ActivationFunctionType.Tanh,
                    bias=bias_sb[:, ub, b : b + 1],
                )
                if ub == 0:
                    nc.vector.tensor_scalar_mul(acc, th, v_sb[:, 0:1])
                else:
                    nc.vector.scalar_tensor_tensor(
                        acc, th, v_sb[:, ub : ub + 1], acc,
                        op0=mybir.AluOpType.mult,
                        op1=mybir.AluOpType.add,
                    )

            # ---- group 0: hb-major over 4-bank passes ---------------------
            # each (x0 chunk, w1 block) DMA pair unlocks one matmul per open
            # bank, so the PE tracks the DMA stream instead of stalling on
            # the full weight load; dummy matmuls fill the leftover gap.
            acc0 = scpool.tile([P, T_GROUP], F32R, tag="acc")
            half_ub = UB // 2
            for p in range(2):
                pus = [psu.tile([P, T_GROUP], F32, tag="pu",
                                name=f"pu0_{p}_{i}")
                       for i in range(half_ub)]
                for kj in range(KJ):
                    for i, pu in enumerate(pus):
                        chain_dr(pu, p * half_ub + i, x80, kj, kj == 0)
                    if p == 0:
                        dummy_mm(2)
                for hb in range(HB):
                    for i, pu in enumerate(pus):
                        chain_bf(pu, p * half_ub + i, x0_tiles, hb)
                    if p == 0 and hb <= HB - 2:
                        dummy_mm(2)
                for i, pu in enumerate(pus):
                    tanh_dve(pu, p * half_ub + i, 0, acc0)
            state["pending"] = (acc0, 0, 0)

            # ---- steady groups 1 .. n-2 -----------------------------------
            for g in range(1, n_groups - 1):
                b = g // groups_per_batch
                gi = g % groups_per_batch
                if g + PREFETCH - 1 < n_groups:
                    issue_x(g + PREFETCH - 1)
                x8t, xT = x_pending.pop(g)
                acc = scpool.tile([P, T_GROUP], F32R, tag="acc")
                for ub in range(UB):
                    pu = psu.tile([P, T_GROUP], F32, tag="pu")
                    for kj in range(KJ):
                        chain_dr(pu, ub, x8t, kj, kj == 0)
                    for hb in range(HB):
                        chain_bf(pu, ub, xT, hb)
                    if ub == 0:
                        # merge of the previous group lands here, after a
                        # full matmul chain has hidden the DVE accumulate
                        pacc, pb, pgi = state["pending"]
                        finish_pe(pacc, pb, pgi)
                    tanh_dve(pu, ub, b, acc)
                state["pending"] = (acc, b, gi)

            # ---- last group: contract V on the PE (short tail) ------------
            g = n_groups - 1
            b = g // groups_per_batch
            gi = g % groups_per_batch
            x8t, xT = x_pending.pop(g)
            mg = None
            th_f = []
            for ub in range(UB):
                pu = psu.tile([P, T_GROUP], F32, tag="pu")
                for kj in range(KJ):
                    chain_dr(pu, ub, x8t, kj, kj == 0)
                for hb in range(HB):
                    chain_bf(pu, ub, xT, hb)
                if ub == 0:
                    pacc, pb, pgi = state["pending"]
                    finish_pe(pacc, pb, pgi)
                    mg = psmg.tile([1, T_GROUP], F32, tag="mg")
                if ub >= 1:
                    # V-matmul for the previous ub (its tanh finished while
                    # this chain ran)
                    nc.tensor.matmul(
                        mg, lhsT=v_r[:, ub - 1 : ub], rhs=th_f[ub - 1],
                        start=(ub == 1), stop=False,
                    )
                th = thpool.tile([P, T_GROUP], F32R, tag="thf")
                if ub == UB - 1:
                    # split the final tanh by token halves so its V-matmul
                    # can start half a tile earlier (shorter drain chain)
                    half_t = T_GROUP // 2
                    for lo in (0, half_t):
                        nc.scalar.activation(
                            th[:, lo : lo + half_t], pu[:, lo : lo + half_t],
                            mybir.ActivationFunctionType.Tanh,
                            bias=bias_sb[:, ub, b : b + 1],
                        )
                else:
                    nc.scalar.activation(
                        th, pu,
                        mybir.ActivationFunctionType.Tanh,
                        bias=bias_sb[:, ub, b : b + 1],
                    )
                th_f.append(th)
            half_t = T_GROUP // 2
            nc.tensor.matmul(
                mg[:, :half_t], lhsT=v_r[:, UB - 1 : UB],
                rhs=th_f[UB - 1][:, :half_t], start=False, stop=False,
            )
            nc.tensor.matmul(
                mg[:, half_t:], lhsT=v_r[:, UB - 1 : UB],
                rhs=th_f[UB - 1][:, half_t:], start=False, stop=True,
            )
            emit_exp(mg, b, gi)

    nc.compile()
    return nc


def make_in_maps(inputs, k8=K8):
    """Shard the full inputs per core: encoder_output pre-rounded (first k8
    H-rows to fp8e4, rest bf16) and pre-transposed to [H, tokens]; the bias
    chain h_n @ W2 + b1 + b2 folded on the host into [u_p, ub*b] f32."""
    import ml_dtypes

    bf16 = ml_dtypes.bfloat16
    e4m3 = ml_dtypes.float8_e4m3

    def f32(name):
        return np.ascontiguousarray(np.asarray(inputs[name], dtype=np.float32))

    enc = f32("encoder_output")
    w1 = f32("W1_w")
    vw = f32("V_w")
    vb = f32("V_b")

    # folded bias: [B, U] = h_n @ W2 + W1_b + W2_b (f32, tiny)
    bias_full = (f32("last_layer_h_n") @ f32("W2_w")
                 + f32("W1_b") + f32("W2_b"))

    UB = U // P
    in_maps = []
    for c in range(N_CORES):
        sl = slice(c * B_LOCAL, (c + 1) * B_LOCAL)
        eT = enc[sl].reshape(B_LOCAL * S, H).T  # [H, tokens] f32
        # [b, U] -> [u_p, ub, b] -> [u_p, ub*b]
        bc = bias_full[sl].reshape(B_LOCAL, UB, P).transpose(2, 1, 0)
        m = {
            "encoder_output": np.ascontiguousarray(eT[k8:]).astype(bf16),
            "W1_w": w1[k8:].astype(bf16),
            "bias_in": np.ascontiguousarray(bc.reshape(P, UB * B_LOCAL)),
            "V_w": vw,
            "V_b": vb,
        }
        if k8:
            m["enc8"] = np.ascontiguousarray(eT[:k8]).astype(e4m3)
            m["W1_8"] = w1[:k8].astype(e4m3)
        in_maps.append(m)
    return in_maps


def kernel(**inputs):
    from concourse.bass_utils import run_bass_kernel_spmd

    nc = build_kernel()
    in_maps = make_in_maps(inputs)
    res = run_bass_kernel_spmd(nc, in_maps, core_ids=list(range(N_CORES)))
    outs = [res.results[c]["out"].reshape(B_LOCAL, S, 1) for c in range(N_CORES)]
    return np.concatenate(outs, axis=0)


# revision 18
# speedup vs baseline: 1.0068x; 1.0068x over previous
"""Bahdanau attention weights kernel for 8 Trainium2 NeuronCores.

Reference computation (per full input):
    proj_enc = encoder_output @ W1_w + W1_b            # [B,S,U]
    proj_h   = last_layer_h_n @ W2_w + W2_b            # [B,1,U]
    score    = tanh(proj_enc + proj_h) @ V_w + V_b     # [B,S,1]
    out      = softmax(score, axis=1)                  # [B,S,1]

Sharding: data-parallel over batch. Each of the 8 cores gets B/8 batches;
weights are replicated; softmax is over the local sequence axis, so no
cross-core communication is needed.

Per-core layout strategy: keep U on partitions.
  - The host pre-transposes encoder_output to [H, tokens] so X^T tiles
    DMA straight into SBUF, and pre-folds the tiny bias chain
    (h_n @ W2 + W1_b + W2_b -> [b, U]) so no W2 traffic or bias matmuls
    hit the device at startup.
  - Main matmuls compute proj^T [u=128, t=512] in PSUM. The first `k8`
    rows of the H contraction run as fp8e4 DoubleRow matmuls (K=256 per
    MM at ~1.4x bf16 FLOP rate); the rest accumulate in bf16. k8=256
    keeps the softmax rel-err ~1.4e-2 (vs 2.2e-3 all-bf16, gate 2e-2).
  - tanh runs on the scalar engine reading PSUM with the folded bias as
    the per-partition bias operand.
  - The V contraction runs on the DVE: acc += V_ub (.) tanh_ub with V as
    a per-partition f32 scalar into an f32r accumulator; one f32r
    all-ones matmul per group sums it over its 128 partitions.
  - Softmax per batch is Exp(accum_out) / reciprocal / scale on the
    [1, 2048] score row (scores are bounded, so no max subtraction).
    The final scale is split across DVE+ACT halves to shorten the tail.
  - Startup: group-0 X chunks and W1 blocks DMA interleaved per h-block
    and group 0 runs hb-major over 4 PSUM banks per pass, so the PE
    streams real matmuls as each chunk pair lands; dummy matmuls warm
    the HAM clock first.
  - The last group contracts V on the PE directly (8 small f32r matmuls
    accumulating into the merge bank) so no DVE chain sits in the tail.
"""

import sys

for _p in ("/opt/trn_rl_repo", "/root/.axon_site/_ro/trn_rl_repo"):
    if _p not in sys.path:
        sys.path.append(_p)

import numpy as np

import concourse.bacc as bacc
import concourse.tile as tile
from concourse import mybir

F32 = mybir.dt.float32
F32R = mybir.dt.float32r
BF16 = mybir.dt.bfloat16
F8E4 = mybir.dt.float8e4
DR = mybir.MatmulPerfMode.DoubleRow

B, S, H, U = 32, 2048, 1024, 1024
N_CORES = 8
B_LOCAL = B // N_CORES  # 4
P = 128
T_GROUP = 512  # tokens per group (matmul moving dim)
K8 = 512       # rows of the H contraction done in fp8 DoubleRow


def build_kernel(b_local=B_LOCAL, s=S, h=H, u=U, k8=K8):
    """Build the per-core Bass program. Shape params must keep:
    s % T_GROUP == 0, h % 128 == 0, u % 128 == 0, k8 % 256 == 0."""
    nc = bacc.Bacc()

    n_tok = b_local * s
    n_groups = n_tok // T_GROUP
    groups_per_batch = s // T_GROUP
    KJ = k8 // 256           # DoubleRow matmuls per chain
    h_bf = h - k8
    HB = h_bf // P           # bf16 h blocks per chain
    UB = u // P              # u blocks
    PREFETCH = 5

    if k8:
        enc8 = nc.dram_tensor("enc8", [k8, n_tok], F8E4, kind="ExternalInput")
        w18 = nc.dram_tensor("W1_8", [k8, u], F8E4, kind="ExternalInput")
        enc8_v = enc8.ap().rearrange(
            "(kj j p) (g t) -> g p kj j t", p=P, j=2, t=T_GROUP)
        w18_v = w18.ap().rearrange("(kj j p) u -> kj p j u", p=P, j=2)
    enc = nc.dram_tensor("encoder_output", [h_bf, n_tok], BF16,
                         kind="ExternalInput")
    w1 = nc.dram_tensor("W1_w", [h_bf, u], BF16, kind="ExternalInput")
    bias_in = nc.dram_tensor("bias_in", [P, UB * b_local], F32,
                             kind="ExternalInput")
    vw = nc.dram_tensor("V_w", [u, 1], F32, kind="ExternalInput")
    vb = nc.dram_tensor("V_b", [1], F32, kind="ExternalInput")
    out = nc.dram_tensor("out", [b_local, s], F32, kind="ExternalOutput")

    encT_v = enc.ap().rearrange("(hb p) (g t) -> g p hb t", p=P, t=T_GROUP)
    w1_v = w1.ap().rearrange("(hb p) u -> hb p u", p=P)

    with tile.TileContext(nc) as tc:
        with (
            tc.tile_pool(name="consts", bufs=1) as consts,
            tc.tile_pool(name="wpool", bufs=1) as wpool,
            tc.tile_pool(name="x0pool", bufs=1) as x0pool,
            tc.tile_pool(name="xtpool", bufs=PREFETCH + 1) as xtpool,
            tc.tile_pool(name="thpool", bufs=4) as thpool,
            tc.tile_pool(name="scpool", bufs=3) as scpool,
            tc.tile_pool(name="rowpool", bufs=2) as rowpool,
            tc.tile_pool(name="smpool", bufs=2) as smpool,
            tc.tile_pool(name="psu", bufs=6, space="PSUM") as psu,
            tc.tile_pool(name="psmg", bufs=1, space="PSUM") as psmg,
            tc.tile_pool(name="pswm", bufs=1, space="PSUM") as pswm,
        ):
            # ---- warm-up fodder (no DMA dependencies) ---------------------
            # one ACT memzero readies both dummy operands (the scalar
            # engine's preamble finishes first); N=1 const-AP matmuls were
            # tried instead and hard-fault the exec unit (NRT 101)
            dummy_x = consts.tile([P, T_GROUP], BF16)
            nc.scalar.memzero(dummy_x)
            warm_ps = pswm.tile([P, T_GROUP], F32, tag="warm")

            def dummy_mm(n):
                for _ in range(n):
                    nc.tensor.matmul(warm_ps, lhsT=dummy_x[:, :P], rhs=dummy_x)

            dummy_mm(5)

            # ---- DMA: group-0 X chunks interleaved with W1 blocks ---------
            # (the critical ~2.6MB: each chunk pair unlocks 4 real matmuls)
            w18_sb = []
            x80 = None
            if k8:
                x80 = x0pool.tile([P, KJ, 2, T_GROUP], F8E4, tag="x8_0")
                for kj in range(KJ):
                    nc.sync.dma_start(out=x80[:, kj], in_=enc8_v[0][:, kj])
                    t8 = wpool.tile([P, 2, u], F8E4, tag=f"w18_{kj}")
                    nc.sync.dma_start(out=t8, in_=w18_v[kj])
                    w18_sb.append(t8)
            x0_tiles = []
            w1_sb = []
            for hb in range(HB):
                xt = x0pool.tile([P, T_GROUP], BF16, tag=f"x0_{hb}")
                nc.sync.dma_start(out=xt, in_=encT_v[0, :, hb, :])
                x0_tiles.append(xt)
                t1 = wpool.tile([P, u], BF16, tag=f"w1b_{hb}")
                nc.sync.dma_start(out=t1, in_=w1_v[hb])
                w1_sb.append(t1)

            # ---- small consts (ride the small-DMA ring) -------------------
            v_sb = consts.tile([P, UB], F32)
            nc.sync.dma_start(
                out=v_sb, in_=vw.ap().rearrange("(ub p) one -> p (ub one)", p=P)
            )
            vb_sb = consts.tile([1, 1], F32)
            nc.sync.dma_start(out=vb_sb, in_=vb.ap().rearrange("(a b) -> a b", a=1))
            bias_sb = consts.tile([P, UB, b_local], F32)
            nc.sync.dma_start(
                out=bias_sb,
                in_=bias_in.ap().rearrange("p (ub b) -> p ub b", b=b_local),
            )

            # all-ones column (f32r): one matmul sums the V-weighted tanh
            # accumulator over its 128 partitions (memset can't emit f32r,
            # so memset f32 and round through a copy)
            ones_f = consts.tile([P, 1], F32)
            nc.vector.memset(ones_f, 1.0)
            ones_sb = consts.tile([P, 1], F32R)
            nc.vector.tensor_copy(ones_sb, ones_f)
            # f32r copy of V for the last group's PE-side V contraction
            v_r = consts.tile([P, UB], F32R)
            nc.vector.tensor_copy(v_r, v_sb)

            # ---- X prefetch for groups 1..PREFETCH-1 ----------------------
            x_pending = {}

            def issue_x(g):
                x8t = None
                if k8:
                    x8t = xtpool.tile([P, KJ, 2, T_GROUP], F8E4, tag="x8")
                    nc.sync.dma_start(out=x8t, in_=enc8_v[g])
                xT = xtpool.tile([P, HB, T_GROUP], BF16, tag="xT")
                nc.sync.dma_start(out=xT, in_=encT_v[g])
                x_pending[g] = (x8t, xT)

            for g0 in range(1, min(PREFETCH, n_groups)):
                issue_x(g0)

            # ---- softmax state & helpers ----------------------------------
            state = {"sc_row": None, "esums": None, "pending": None}

            def finish_batch(pb):
                sc_row, esums = state["sc_row"], state["esums"]
                esum = smpool.tile([1, 1], F32, tag="esum")
                nc.vector.tensor_reduce(
                    esum, esums, axis=mybir.AxisListType.X,
                    op=mybir.AluOpType.add,
                )
                rec = smpool.tile([1, 1], F32, tag="rec")
                nc.vector.reciprocal(rec, esum)
                # scale the row split DVE/ACT by their measured rates (DVE
                # ~1.4 elem/ns vs ACT ~0.83 single-partition), sliced so
                # each output DMA starts as soon as its slice is scaled
                cut = (s * 21) // 32
                q = cut // 2
                for lo, hi in ((0, q), (q, cut)):
                    nc.vector.tensor_scalar_mul(
                        sc_row[:, lo:hi], sc_row[:, lo:hi], rec)
                    nc.sync.dma_start(out=out.ap()[pb : pb + 1, lo:hi],
                                      in_=sc_row[:, lo:hi])
                nc.scalar.mul(sc_row[:, cut:], sc_row[:, cut:], rec)
                nc.sync.dma_start(out=out.ap()[pb : pb + 1, cut:],
                                  in_=sc_row[:, cut:])

            def emit_exp(score_ps, pb, pgi):
                # score chunk -> exp incrementally per chunk (adds V_b).
                # scores are bounded (|score| <= sum|V_w|+|V_b| < 17), so
                # exp without max-subtraction is safe in fp32.
                if pgi == 0:
                    state["sc_row"] = rowpool.tile(
                        [1, s], F32, tag="scrow", name="sc_row")
                    state["esums"] = smpool.tile(
                        [1, groups_per_batch], F32, tag="esums", name="esums")
                sc_row, esums = state["sc_row"], state["esums"]
                nc.scalar.activation(
                    sc_row[:, pgi * T_GROUP : (pgi + 1) * T_GROUP], score_ps,
                    mybir.ActivationFunctionType.Exp,
                    bias=vb_sb,
                    accum_out=esums[:, pgi : pgi + 1],
                )
                if pgi == groups_per_batch - 1:
                    finish_batch(pb)

            def finish_pe(acc, pb, pgi):
                # merge: f32r ones-matmul reads the f32r accumulator
                # (a GPSIMD partition_all_reduce was tried instead: ~3.5us
                # per group and it stalls the PE ~1us/group — net loss)
                score_ps = psmg.tile([1, T_GROUP], F32, tag="mg")
                nc.tensor.matmul(score_ps, lhsT=ones_sb, rhs=acc)
                emit_exp(score_ps, pb, pgi)

            def chain_dr(pu, ub, x8t, kj, start):
                nc.tensor.matmul(
                    pu,
                    lhsT=w18_sb[kj][:, :, ub * P : (ub + 1) * P],
                    rhs=x8t[:, kj],
                    start=start, stop=False,
                    perf_mode=DR,
                )

            def chain_bf(pu, ub, xbt, hb):
                nc.tensor.matmul(
                    pu,
                    lhsT=w1_sb[hb][:, ub * P : (ub + 1) * P],
                    rhs=xbt[hb] if isinstance(xbt, list) else xbt[:, hb, :],
                    start=(k8 == 0 and hb == 0),
                    stop=(hb == HB - 1),
                )

            def tanh_dve(pu, ub, b, acc):
                th = thpool.tile([P, T_GROUP], BF16, tag="th")
                nc.scalar.activation(
                    th, pu,
                    mybir.ActivationFunctionType.Tanh,
                    bias=bias_sb[:, ub, b : b + 1],
                )
                if ub == 0:
                    nc.vector.tensor_scalar_mul(acc, th, v_sb[:, 0:1])
                else:
                    nc.vector.scalar_tensor_tensor(
                        acc, th, v_sb[:, ub : ub + 1], acc,
                        op0=mybir.AluOpType.mult,
                        op1=mybir.AluOpType.add,
                    )

            # ---- group 0: hb-major over 4-bank passes ---------------------
            # each (x0 chunk, w1 block) DMA pair unlocks one matmul per open
            # bank, so the PE tracks the DMA stream instead of stalling on
            # the full weight load; dummy matmuls fill the leftover gap.
            acc0 = scpool.tile([P, T_GROUP], F32R, tag="acc")
            half_ub = UB // 2
            for p in range(2):
                pus = [psu.tile([P, T_GROUP], F32, tag="pu",
                                name=f"pu0_{p}_{i}")
                       for i in range(half_ub)]
                for kj in range(KJ):
                    for i, pu in enumerate(pus):
                        chain_dr(pu, p * half_ub + i, x80, kj, kj == 0)
                    if p == 0:
                        dummy_mm(2)
                for hb in range(HB):
                    for i, pu in enumerate(pus):
                        chain_bf(pu, p * half_ub + i, x0_tiles, hb)
                    if p == 0 and hb <= HB - 2:
                        dummy_mm(2)
                for i, pu in enumerate(pus):
                    tanh_dve(pu, p * half_ub + i, 0, acc0)
            state["pending"] = (acc0, 0, 0)

            # ---- steady groups 1 .. n-2 -----------------------------------
            for g in range(1, n_groups - 1):
                b = g // groups_per_batch
                gi = g % groups_per_batch
                if g + PREFETCH - 1 < n_groups:
                    issue_x(g + PREFETCH - 1)
                x8t, xT = x_pending.pop(g)
                acc = scpool.tile([P, T_GROUP], F32R, tag="acc")
                for ub in range(UB):
                    pu = psu.tile([P, T_GROUP], F32, tag="pu")
                    for kj in range(KJ):
                        chain_dr(pu, ub, x8t, kj, kj == 0)
                    for hb in range(HB):
                        chain_bf(pu, ub, xT, hb)
                    if ub == 0:
                        # merge of the previous group lands here, after a
                        # full matmul chain has hidden the DVE accumulate
                        pacc, pb, pgi = state["pending"]
                        finish_pe(pacc, pb, pgi)
                    tanh_dve(pu, ub, b, acc)
                state["pending"] = (acc, b, gi)

            # ---- last group: contract V on the PE (short tail) ------------
            g = n_groups - 1
            b = g // groups_per_batch
            gi = g % groups_per_batch
            x8t, xT = x_pending.pop(g)
            mg = None
            th_f = []
            half_t = T_GROUP // 2
            for ub in range(UB):
                pu = psu.tile([P, T_GROUP], F32, tag="pu")
                if ub < UB - 1:
                    for kj in range(KJ):
                        chain_dr(pu, ub, x8t, kj, kj == 0)
                    for hb in range(HB):
                        chain_bf(pu, ub, xT, hb)
                    if ub == 0:
                        pacc, pb, pgi = state["pending"]
                        finish_pe(pacc, pb, pgi)
                        mg = psmg.tile([1, T_GROUP], F32, tag="mg")
                    else:
                        # V-matmul for the previous ub (its tanh finished
                        # while this chain ran)
                        nc.tensor.matmul(
                            mg, lhsT=v_r[:, ub - 1 : ub], rhs=th_f[ub - 1],
                            start=(ub == 1), stop=False,
                        )
                    th = thpool.tile([P, T_GROUP], F32R, tag="thf")
                    nc.scalar.activation(
                        th, pu,
                        mybir.ActivationFunctionType.Tanh,
                        bias=bias_sb[:, ub, b : b + 1],
                    )
                    th_f.append(th)
                else:
                    # final ub: half-token sub-chains so each tanh half
                    # overlaps the other half's matmuls — shortest drain
                    th = thpool.tile([P, T_GROUP], F32R, tag="thf")
                    for li, lo in enumerate((0, half_t)):
                        for kj in range(KJ):
                            nc.tensor.matmul(
                                pu[:, lo : lo + half_t],
                                lhsT=w18_sb[kj][:, :, ub * P : (ub + 1) * P],
                                rhs=x8t[:, kj, :, lo : lo + half_t],
                                start=(kj == 0), stop=False, perf_mode=DR,
                            )
                        for hb in range(HB):
                            nc.tensor.matmul(
                                pu[:, lo : lo + half_t],
                                lhsT=w1_sb[hb][:, ub * P : (ub + 1) * P],
                                rhs=xT[:, hb, lo : lo + half_t],
                                start=False, stop=(hb == HB - 1),
                            )
                        if li == 0:
                            nc.tensor.matmul(
                                mg, lhsT=v_r[:, ub - 1 : ub],
                                rhs=th_f[ub - 1], start=False, stop=False,
                            )
                        else:
                            nc.tensor.matmul(
                                mg[:, :half_t], lhsT=v_r[:, ub : ub + 1],
                                rhs=th[:, :half_t], start=False, stop=False,
                            )
                        nc.scalar.activation(
                            th[:, lo : lo + half_t], pu[:, lo : lo + half_t],
                            mybir.ActivationFunctionType.Tanh,
                            bias=bias_sb[:, ub, b : b + 1],
                        )
                    th_f.append(th)
            nc.tensor.matmul(
                mg[:, half_t:], lhsT=v_r[:, UB - 1 : UB],
                rhs=th_f[UB - 1][:, half_t:], start=False, stop=True,
            )
            emit_exp(mg, b, gi)

    nc.compile()
    return nc


def make_in_maps(inputs, k8=K8):
    """Shard the full inputs per core: encoder_output pre-rounded (first k8
    H-rows to fp8e4, rest bf16) and pre-transposed to [H, tokens]; the bias
    chain h_n @ W2 + b1 + b2 folded on the host into [u_p, ub*b] f32."""
    import ml_dtypes

    bf16 = ml_dtypes.bfloat16
    e4m3 = ml_dtypes.float8_e4m3

    def f32(name):
        return np.ascontiguousarray(np.asarray(inputs[name], dtype=np.float32))

    enc = f32("encoder_output")
    w1 = f32("W1_w")
    vw = f32("V_w")
    vb = f32("V_b")

    # folded bias: [B, U] = h_n @ W2 + W1_b + W2_b (f32, tiny)
    bias_full = (f32("last_layer_h_n") @ f32("W2_w")
                 + f32("W1_b") + f32("W2_b"))

    UB = U // P
    in_maps = []
    for c in range(N_CORES):
        sl = slice(c * B_LOCAL, (c + 1) * B_LOCAL)
        eT = enc[sl].reshape(B_LOCAL * S, H).T  # [H, tokens] f32
        # [b, U] -> [u_p, ub, b] -> [u_p, ub*b]
        bc = bias_full[sl].reshape(B_LOCAL, UB, P).transpose(2, 1, 0)
        m = {
            "encoder_output": np.ascontiguousarray(eT[k8:]).astype(bf16),
            "W1_w": w1[k8:].astype(bf16),
            "bias_in": np.ascontiguousarray(bc.reshape(P, UB * B_LOCAL)),
            "V_w": vw,
            "V_b": vb,
        }
        if k8:
            m["enc8"] = np.ascontiguousarray(eT[:k8]).astype(e4m3)
            m["W1_8"] = w1[:k8].astype(e4m3)
        in_maps.append(m)
    return in_maps


def kernel(**inputs):
    from concourse.bass_utils import run_bass_kernel_spmd

    nc = build_kernel()
    in_maps = make_in_maps(inputs)
    res = run_bass_kernel_spmd(nc, in_maps, core_ids=list(range(N_CORES)))
    outs = [res.results[c]["out"].reshape(B_LOCAL, S, 1) for c in range(N_CORES)]
    return np.concatenate(outs, axis=0)
